# revision 1
# baseline (speedup 1.0000x reference)
"""Trainium2 Bass kernel for nn_CompressedInteractionNet_31997506355236.

Reference math (per batch b, channel k, dim d; m == H == 64, D == 16, vk == 16):
    x0r[b,d,:]  = x_0[b,:,d]                      # [m]
    xhr[b,d,:]  = x_0[b].reshape(D, H)[d]         # [H] (flat reinterpretation)
    out[b,k,d]  = sum_v (x0r[b,d] @ Vm[k,0,:,v]) * (Vh[k,0,v,:] @ xhr[b,d])

Strategy: 2D sharding, batch x channels = 4 x 2 over 8 cores (32 batches and
32 output channels per core) — minimizes per-core DMA bytes at equal compute.
Host-side sharding lays the operands out so every device DMA is fully
contiguous (DMA engines are packet/descriptor-rate-bound; strided 64B-run
loads are ~10x slower):
    xc  [m, 2*bd]  = [x0t | xhrt]  (both lhsT operands, per batch shard)
    vmf [m, 512], vhf [j, 512]     (rhs operands, per k shard)
Device, per 128-row chunk c (4 units):
    A = x0t_c.T @ vmf, Bt = xhrt_c.T @ vhf      (PE, f32r, PSUM)
    b_sb = copy(Bt)                             (ACT; DVE allows <=1 PSUM input)
    P = A * b_sb                                (DVE)
    O[bd, k] = sum_v P[bd, k, v]                (GPSIMD half-add + DVE reduce;
                                                 last unit all-DVE)
Output leaves the device as [(b,d), k_loc]; the host unshards and transposes
back to [B, Hk, D].
"""

import numpy as np

import concourse.bass as bass
import concourse.tile as tile
from concourse import bacc, mybir
from concourse.bass_utils import run_bass_kernel_spmd

# Problem constants (hardcoded; kernel must be self-contained).
B, M, D = 128, 64, 16
HK, VK = 64, 16
H = 64
NCORES = 8
SB, SK = 4, 2             # batch shards x channel shards
BL = B // SB              # batches per core = 32
BD = BL * D               # rows per core = 512
KL = HK // SK             # channels per core = 32
KVL = KL * VK             # 512
NCH = BD // 128           # 128-row chunks per core = 4
F32 = mybir.dt.float32
F32R = mybir.dt.float32r

_CACHE = {}


def build_bass():
    nc = bacc.Bacc("TRN2", target_bir_lowering=False, debug=False,
                   num_devices=NCORES, enable_partition_id=False,
                   monotonic_sem_count=0)

    # xc piece p holds [x0t chunks 2p,2p+1 | xhrt chunks 2p,2p+1]
    xc0_d = nc.dram_tensor("xc0", [M, BD], F32, kind="ExternalInput")
    xc1_d = nc.dram_tensor("xc1", [M, BD], F32, kind="ExternalInput")
    vmf_d = nc.dram_tensor("vmf", [M, KVL], F32, kind="ExternalInput")
    vhf_d = nc.dram_tensor("vhf", [H, KVL], F32, kind="ExternalInput")
    out = nc.dram_tensor("out", [BD, KL], F32, kind="ExternalOutput")

    with tile.TileContext(nc) as tc:
        with (
            tc.tile_pool(name="w", bufs=1) as w,
            tc.tile_pool(name="work", bufs=3) as work,
            tc.tile_pool(name="pab", bufs=2, space="PSUM") as pab,
        ):
            # ---- contiguous loads spread over the 3 issue queues -------
            vhf = w.tile([H, KVL], F32R)
            nc.sync.dma_start(vhf[:], vhf_d.ap().bitcast(F32R))
            xc0 = w.tile([M, BD], F32R)
            nc.scalar.dma_start(xc0[:], xc0_d.ap().bitcast(F32R))
            vmf = w.tile([M, KVL], F32R)
            nc.gpsimd.dma_start(vmf[:], vmf_d.ap().bitcast(F32R))
            xc1 = w.tile([M, BD], F32R)
            nc.gpsimd.dma_start(xc1[:], xc1_d.ap().bitcast(F32R))
            xcs = [xc0, xc1]

            def unit(c, last):
                xp = xcs[c // 2]
                off = (c % 2) * 128
                psum_b = pab.tile([128, KVL], F32, tag="b")
                nc.tensor.matmul(psum_b[:], xp[:, 256 + off:384 + off], vhf[:],
                                 start=True, stop=True)
                psum_a = pab.tile([128, KVL], F32, tag="a")
                nc.tensor.matmul(psum_a[:], xp[:, off:128 + off], vmf[:],
                                 start=True, stop=True)

                b_sb = work.tile([128, KL, VK], F32, tag="b_sb")
                nc.scalar.copy(b_sb.rearrange("p k v -> p (k v)"), psum_b[:])
                p_sb = work.tile([128, KL, VK], F32, tag="p_sb")
                nc.vector.tensor_mul(
                    out=p_sb.rearrange("p k v -> p (k v)"),
                    in0=psum_a[:],
                    in1=b_sb.rearrange("p k v -> p (k v)"))
                o_sb = work.tile([128, KL], F32, tag="o_sb")
                if last:
                    # shortest tail chain: direct DVE reduce over v=16
                    nc.vector.tensor_reduce(out=o_sb[:], in_=p_sb[:],
                                            axis=mybir.AxisListType.X,
                                            op=mybir.AluOpType.add)
                else:
                    # GPSIMD folds v 16->8, DVE reduces the rest
                    t1 = work.tile([128, KL, VK // 2], F32, tag="t1")
                    nc.gpsimd.tensor_tensor(t1[:], p_sb[:, :, 0:8],
                                            p_sb[:, :, 8:16],
                                            mybir.AluOpType.add)
                    nc.vector.tensor_reduce(out=o_sb[:], in_=t1[:],
                                            axis=mybir.AxisListType.X,
                                            op=mybir.AluOpType.add)
                nc.sync.dma_start(out.ap()[128 * c:128 * (c + 1), :], o_sb[:])

            for c in range(NCH):
                unit(c, last=(c == NCH - 1))

    nc.compile()
    return nc


def run(x_0, x_h, Vm, Vh, **spmd_kwargs):
    x_0 = np.ascontiguousarray(np.asarray(x_0), dtype=np.float32)
    vm = np.asarray(Vm)[:, 0].astype(np.float32)
    vh = np.asarray(Vh)[:, 0].astype(np.float32)

    # Host-side layout prep (part of sharding): all-contiguous device inputs.
    vmf = np.ascontiguousarray(vm.transpose(1, 0, 2).reshape(M, HK * VK))
    vhf = np.ascontiguousarray(vh.transpose(2, 0, 1).reshape(H, HK * VK))

    if "nc" not in _CACHE:
        _CACHE["nc"] = build_bass()
    nc = _CACHE["nc"]

    in_maps = []
    for core in range(NCORES):
        cb, ck = divmod(core, SK)
        shard = x_0[BL * cb:BL * (cb + 1)]                    # [BL, M, D]
        x0t = shard.transpose(1, 0, 2).reshape(M, BD)         # [i, (b,d)]
        xhrt = shard.reshape(BL, D, H).transpose(2, 0, 1).reshape(H, BD)
        xc0 = np.ascontiguousarray(
            np.concatenate([x0t[:, 0:256], xhrt[:, 0:256]], axis=1))
        xc1 = np.ascontiguousarray(
            np.concatenate([x0t[:, 256:512], xhrt[:, 256:512]], axis=1))
        ks = slice(KVL * ck, KVL * (ck + 1))
        in_maps.append({
            "xc0": xc0,
            "xc1": xc1,
            "vmf": np.ascontiguousarray(vmf[:, ks]),
            "vhf": np.ascontiguousarray(vhf[:, ks]),
        })

    res = run_bass_kernel_spmd(nc, in_maps, core_ids=list(range(NCORES)),
                               **spmd_kwargs)
    # Unshard: per-core out is [(b,d), k_loc] -> [BL, D, KL] -> [BL, KL, D]
    full = np.empty((B, HK, D), dtype=np.float32)
    for core in range(NCORES):
        cb, ck = divmod(core, SK)
        o = res.results[core]["out"].reshape(BL, D, KL).transpose(0, 2, 1)
        full[BL * cb:BL * (cb + 1), KL * ck:KL * (ck + 1), :] = o
    return full, res


def kernel(x_0, x_h, Vm, Vh):
    return run(x_0, x_h, Vm, Vh)[0]


if __name__ == "__main__":
    rng = np.random.default_rng(0)
    x_0 = rng.standard_normal((B, M, D)).astype(np.float32)
    x_h = rng.standard_normal((B, H, D)).astype(np.float32)
    Vm = rng.standard_normal((HK, 1, M, VK)).astype(np.float32)
    Vh = rng.standard_normal((HK, 1, VK, H)).astype(np.float32)
    got = kernel(x_0, x_h, Vm, Vh)

    x0r = np.transpose(x_0, (0, 2, 1))
    xhr = x_0.reshape(B, D, H)
    a = np.einsum("bdi,kiv->bkdv", x0r, Vm[:, 0])
    bb = np.einsum("bdj,kvj->bkdv", xhr, Vh[:, 0])
    want = np.einsum("bkdv,bkdv->bkd", a, bb)
    err = np.abs(got - want).max() / np.abs(want).max()
    print("rel err:", err)



# revision 5
# speedup vs baseline: 1.0245x; 1.0245x over previous
"""Trainium2 Bass kernel for nn_CompressedInteractionNet_31997506355236.

Reference math (per batch b, channel k, dim d; m == H == 64, D == 16, vk == 16):
    x0r[b,d,:]  = x_0[b,:,d]                      # [m]
    xhr[b,d,:]  = x_0[b].reshape(D, H)[d]         # [H] (flat reinterpretation)
    out[b,k,d]  = sum_v (x0r[b,d] @ Vm[k,0,:,v]) * (Vh[k,0,v,:] @ xhr[b,d])

Sharding: 2D, batch x channels = 4 x 2 over 8 cores (BL=32 batches, KL=32
output channels per core) -- minimizes per-core DMA bytes.

v2 design (vs f32r baseline):
  * all operands bf16 (host casts; tolerance is 2e-2, bf16 keeps ~0.5% err)
  * transposed matmul orientation: products land as [kv, bd] in PSUM
        psum_a[kv,bd] = vmf_chunk.T @ x0t      (lhsT=vmf [64,128], rhs=x0t)
        psum_b[kv,bd] = vhf_chunk.T @ xhrt
    so the v-reduction (groups of 16 along kv = partitions) is done on the
    PE with a 0/1 select matrix:  psum_o += sel_c.T @ p2_c  -- no DVE
    reduce at all.
  * per kv-chunk c (4 chunks of 128 kv = 8 k):
        ACT:  b2 = copy(psum_b)            f32->f32 SBUF
        DVE:  p2 = psum_a * b2  -> bf16    (one PSUM operand)
        PE:   psum_o[32,cols] += sel_c.T @ p2_c   (split in two bd-column
              halves A/B so the first egress+store overlaps the tail)
  * PE warm-up: dummy matmuls on a memset tile run during the input-DMA
    wait so the HAM clock gate reaches 2.4 GHz before the real matmuls.
  * 2 input DMAs, 128 partitions, 1-1.25KB/partition lines (HWDGE rings).
  * bf16 output, host casts back to float32.
"""

import numpy as np
import ml_dtypes

import concourse.bass as bass
import concourse.tile as tile
from concourse import bacc, mybir
from concourse.bass_utils import run_bass_kernel_spmd

# Problem constants (hardcoded; kernel must be self-contained).
B, M, D = 128, 64, 16
HK, VK = 64, 16
H = 64
NCORES = 8
SB, SK = 4, 2             # batch shards x channel shards
BL = B // SB              # batches per core = 32
BD = BL * D               # bd columns per core = 512
KL = HK // SK             # channels per core = 32
KVL = KL * VK             # kv rows per core = 512
NCH = KVL // 128          # 128-row kv chunks per core = 4
NDUMMY = 5                # PE warm-up matmuls during DMA wait
F32 = mybir.dt.float32
BF16 = mybir.dt.bfloat16
BF = ml_dtypes.bfloat16

XCOLS = BD + 4 * KL       # 512 data + 128 sel columns

_CACHE = {}


def build_bass():
    nc = bacc.Bacc("TRN2", target_bir_lowering=False, debug=False,
                   num_devices=NCORES, enable_partition_id=False,
                   monotonic_sem_count=0)

    # xin: rows 0-63 x0t[m, bd], rows 64-127 xhrt[h, bd];
    #      cols 512.. : sel matrices (4 chunks x 32 cols, 0/1 entries)
    xin_d = nc.dram_tensor("xin", [128, XCOLS], BF16, kind="ExternalInput")
    # vin: rows 0-63 vmf[m, kv], rows 64-127 vhf[h, kv]
    vin_d = nc.dram_tensor("vin", [128, KVL], BF16, kind="ExternalInput")
    # out: rows 0-31 = [k_loc, bd 0:256], rows 32-63 = [k_loc, bd 256:512]
    out_d = nc.dram_tensor("out", [2 * KL, BD // 2], BF16,
                           kind="ExternalOutput")

    with tile.TileContext(nc) as tc:
        with (
            tc.tile_pool(name="w", bufs=1) as w,
            tc.tile_pool(name="work", bufs=3) as work,
            tc.tile_pool(name="pa", bufs=2, space="PSUM") as pa,
            tc.tile_pool(name="pb", bufs=2, space="PSUM") as pb,
            tc.tile_pool(name="po", bufs=2, space="PSUM") as po,
        ):
            dmy = w.tile([64, 528], BF16)
            nc.gpsimd.memset(dmy[:], 0.0)

            xin = w.tile([128, XCOLS], BF16)
            nc.sync.dma_start(xin[:], xin_d.ap())
            vin = w.tile([128, KVL], BF16)
            nc.scalar.dma_start(vin[:], vin_d.ap())

            # full-bank [32, 512] tiles; A uses cols 0:256, B cols 256:512
            po_a = po.tile([KL, BD], F32, tag="oa")
            po_b = po.tile([KL, BD], F32, tag="ob")

            # PE warm-up: junk matmuls, data-independent, run during the
            # input DMA wait; r0 (start=True) later resets the bank.
            for _ in range(NDUMMY):
                nc.tensor.matmul(po_a[0:16, :], dmy[:, 0:16], dmy[:, 16:528],
                                 start=True, stop=True)

            for c in range(NCH):
                ksl = slice(128 * c, 128 * (c + 1))
                psum_b = pb.tile([128, BD], F32, tag="b")
                nc.tensor.matmul(psum_b[:], vin[64:128, ksl], xin[64:128, 0:BD],
                                 start=True, stop=True)
                psum_a = pa.tile([128, BD], F32, tag="a")
                nc.tensor.matmul(psum_a[:], vin[0:64, ksl], xin[0:64, 0:BD],
                                 start=True, stop=True)

                b2 = work.tile([128, BD], F32, tag="b2")
                nc.scalar.copy(b2[:], psum_b[:])
                p2 = work.tile([128, BD], BF16, tag="p2")
                nc.vector.tensor_mul(out=p2[:], in0=psum_a[:], in1=b2[:])

                sel = xin[:, BD + 32 * c: BD + 32 * (c + 1)]
                nc.tensor.matmul(po_a[:, 0:256], sel, p2[:, 0:256],
                                 start=(c == 0), stop=(c == NCH - 1))
                nc.tensor.matmul(po_b[:, 256:512], sel, p2[:, 256:512],
                                 start=(c == 0), stop=(c == NCH - 1))

            o_a = work.tile([KL, 256], BF16, tag="oa")
            nc.scalar.copy(o_a[:], po_a[:, 0:256])
            nc.sync.dma_start(out_d.ap()[0:KL, :], o_a[:])
            o_b = work.tile([KL, 256], BF16, tag="ob")
            nc.vector.tensor_copy(o_b[:], po_b[:, 256:512])
            nc.sync.dma_start(out_d.ap()[KL:2 * KL, :], o_b[:])

    nc.compile()
    return nc


def _host_prep(x_0, Vm, Vh):
    """Per-core input blobs: xin [8][128, XCOLS] bf16, vin [8][128, KVL]."""
    x_0 = np.ascontiguousarray(np.asarray(x_0), dtype=np.float32)
    vm = np.asarray(Vm)[:, 0].astype(np.float32)     # [HK, M, VK]
    vh = np.asarray(Vh)[:, 0].astype(np.float32)     # [HK, VK, H]

    vmf = vm.transpose(1, 0, 2).reshape(M, HK * VK)  # [m, (k,v)]
    vhf = vh.transpose(2, 0, 1).reshape(H, HK * VK)  # [h, (k,v)]

    # sel[c][p, j] = 1 iff j == 8*c + p//16   (kv partition p -> k column)
    sel = np.zeros((128, 4 * KL), dtype=np.float32)
    for c in range(NCH):
        for p in range(128):
            sel[p, 32 * c + 8 * c + p // 16] = 1.0

    in_maps = []
    for core in range(NCORES):
        cb, ck = divmod(core, SK)
        shard = x_0[BL * cb:BL * (cb + 1)]                    # [BL, M, D]
        x0t = shard.transpose(1, 0, 2).reshape(M, BD)         # [m, (b,d)]
        xhrt = shard.reshape(BL, D, H).transpose(2, 0, 1).reshape(H, BD)
        xin = np.empty((128, XCOLS), dtype=BF)
        xin[0:64, 0:BD] = x0t.astype(BF)
        xin[64:128, 0:BD] = xhrt.astype(BF)
        xin[:, BD:] = sel.astype(BF)
        ks = slice(KVL * ck, KVL * (ck + 1))
        vin = np.empty((128, KVL), dtype=BF)
        vin[0:64] = vmf[:, ks].astype(BF)
        vin[64:128] = vhf[:, ks].astype(BF)
        in_maps.append({"xin": np.ascontiguousarray(xin),
                        "vin": np.ascontiguousarray(vin)})
    return in_maps


def run(x_0, x_h, Vm, Vh, **spmd_kwargs):
    in_maps = _host_prep(x_0, Vm, Vh)
    if "nc" not in _CACHE:
        _CACHE["nc"] = build_bass()
    nc = _CACHE["nc"]

    res = run_bass_kernel_spmd(nc, in_maps, core_ids=list(range(NCORES)),
                               **spmd_kwargs)
    # Unshard: per-core out is [k_loc, (b,d)] bf16 -> [BL, KL, D] f32
    full = np.empty((B, HK, D), dtype=np.float32)
    for core in range(NCORES):
        cb, ck = divmod(core, SK)
        o2 = np.asarray(res.results[core]["out"]).astype(np.float32)
        o = np.concatenate([o2[0:KL], o2[KL:2 * KL]], axis=1)  # [KL, BD]
        o = o.reshape(KL, BL, D).transpose(1, 0, 2)           # [BL, KL, D]
        full[BL * cb:BL * (cb + 1), KL * ck:KL * (ck + 1), :] = o
    return full, res


def kernel(x_0, x_h, Vm, Vh):
    return run(x_0, x_h, Vm, Vh)[0]


if __name__ == "__main__":
    rng = np.random.default_rng(0)
    x_0 = rng.standard_normal((B, M, D)).astype(np.float32)
    x_h = rng.standard_normal((B, H, D)).astype(np.float32)
    Vm = rng.standard_normal((HK, 1, M, VK)).astype(np.float32)
    Vh = rng.standard_normal((HK, 1, VK, H)).astype(np.float32)
    got = kernel(x_0, x_h, Vm, Vh)

    x0r = np.transpose(x_0, (0, 2, 1))
    xhr = x_0.reshape(B, D, H)
    a = np.einsum("bdi,kiv->bkdv", x0r, Vm[:, 0])
    bb = np.einsum("bdj,kvj->bkdv", xhr, Vh[:, 0])
    want = np.einsum("bkdv,bkdv->bkd", a, bb)
    err = np.abs(got - want).max() / np.abs(want).max()
    print("rel err:", err)


# revision 9
# speedup vs baseline: 1.0788x; 1.0530x over previous
"""Trainium2 Bass kernel for nn_CompressedInteractionNet_31997506355236.

Reference math (per batch b, channel k, dim d; m == H == 64, D == 16, vk == 16):
    x0r[b,d,:]  = x_0[b,:,d]                      # [m]
    xhr[b,d,:]  = x_0[b].reshape(D, H)[d]         # [H] (flat reinterpretation)
    out[b,k,d]  = sum_v (x0r[b,d] @ Vm[k,0,:,v]) * (Vh[k,0,v,:] @ xhr[b,d])

Sharding: 2D, batch x channels = 4 x 2 over 8 cores (BL=32 batches, KL=32
output channels per core) -- minimizes per-core DMA bytes.

v2 design (vs f32r baseline):
  * all operands bf16 (host casts; tolerance is 2e-2, bf16 keeps ~0.5% err)
  * transposed matmul orientation: products land as [kv, bd] in PSUM
        psum_a[kv,bd] = vmf_chunk.T @ x0t      (lhsT=vmf [64,128], rhs=x0t)
        psum_b[kv,bd] = vhf_chunk.T @ xhrt
    so the v-reduction (groups of 16 along kv = partitions) is done on the
    PE with a 0/1 select matrix:  psum_o += sel_c.T @ p2_c  -- no DVE
    reduce at all.
  * per kv-chunk c (4 chunks of 128 kv = 8 k):
        ACT:  b2 = copy(psum_b)            f32->f32 SBUF
        DVE:  p2 = psum_a * b2  -> bf16    (one PSUM operand)
        PE:   psum_o[32,cols] += sel_c.T @ p2_c   (split in two bd-column
              halves A/B so the first egress+store overlaps the tail)
  * PE warm-up: dummy matmuls on a memset tile run during the input-DMA
    wait so the HAM clock gate reaches 2.4 GHz before the real matmuls.
  * 2 input DMAs, 128 partitions, 1-1.25KB/partition lines (HWDGE rings).
  * bf16 output, host casts back to float32.
"""

import numpy as np
import ml_dtypes

import concourse.bass as bass
import concourse.tile as tile
from concourse import bacc, mybir
from concourse.bass_utils import run_bass_kernel_spmd

# Problem constants (hardcoded; kernel must be self-contained).
B, M, D = 128, 64, 16
HK, VK = 64, 16
H = 64
NCORES = 8
SB, SK = 4, 2             # batch shards x channel shards
BL = B // SB              # batches per core = 32
BD = BL * D               # bd columns per core = 512
KL = HK // SK             # channels per core = 32
KVL = KL * VK             # kv rows per core = 512
NCH = KVL // 128          # 128-row kv chunks per core = 4
NDUMMY = 6                # PE warm-up matmuls during DMA wait
F32 = mybir.dt.float32
BF16 = mybir.dt.bfloat16
BF = ml_dtypes.bfloat16

_CACHE = {}


def build_bass():
    nc = bacc.Bacc("TRN2", target_bir_lowering=False, debug=False,
                   num_devices=NCORES, enable_partition_id=False,
                   monotonic_sem_count=0)

    # xv: rows 0-63 [x0t | vmf], rows 64-127 [xhrt | vhf] -- one DMA with
    # 2KB partition lines; both matmul operand pairs share a partition base.
    xv_d = nc.dram_tensor("xv", [128, 2 * BD], BF16, kind="ExternalInput")
    # sel matrices (4 chunks x 32 cols, 0/1 entries), small side DMA
    sel_d = nc.dram_tensor("sel", [128, 4 * KL], BF16, kind="ExternalInput")
    # out: rows 0-31 = [k_loc, bd 0:256], rows 32-63 = [k_loc, bd 256:512]
    out_d = nc.dram_tensor("out", [2 * KL, BD // 2], BF16,
                           kind="ExternalOutput")

    with tile.TileContext(nc) as tc:
        with (
            tc.tile_pool(name="w", bufs=1) as w,
            tc.tile_pool(name="work", bufs=3) as work,
            tc.tile_pool(name="pa", bufs=3, space="PSUM") as pa,
            tc.tile_pool(name="pb", bufs=3, space="PSUM") as pb,
            tc.tile_pool(name="po", bufs=1, space="PSUM") as po,
        ):
            dmy = w.tile([64, 528], BF16)
            nc.gpsimd.memset(dmy[:], 0.0)

            xv = w.tile([128, 2 * BD], BF16)
            nc.sync.dma_start(xv[:], xv_d.ap())
            selt = w.tile([128, 4 * KL], BF16)
            nc.gpsimd.dma_start(selt[:], sel_d.ap())

            # full-bank [32, 512] tiles; A uses cols 0:256, B cols 256:512
            po_a = po.tile([KL, BD], F32, tag="oa")
            po_b = po.tile([KL, BD], F32, tag="ob")

            # PE warm-up: junk matmuls, data-independent, run during the
            # input DMA wait; r0 (start=True) later resets the bank.
            for _ in range(NDUMMY):
                nc.tensor.matmul(po_a[0:16, :], dmy[:, 0:16], dmy[:, 16:528],
                                 start=True, stop=True)

            for c in range(NCH):
                vsl = slice(BD + 128 * c, BD + 128 * (c + 1))
                psum_b = pb.tile([128, BD], F32, tag="b")
                nc.tensor.matmul(psum_b[:], xv[64:128, vsl], xv[64:128, 0:BD],
                                 start=True, stop=True)
                psum_a = pa.tile([128, BD], F32, tag="a")
                nc.tensor.matmul(psum_a[:], xv[0:64, vsl], xv[0:64, 0:BD],
                                 start=True, stop=True)

                b2 = work.tile([128, BD], F32, tag="b2")
                nc.scalar.copy(b2[:], psum_b[:])
                p2 = work.tile([128, BD], BF16, tag="p2")
                nc.vector.tensor_mul(out=p2[:], in0=psum_a[:], in1=b2[:])

                sel = selt[:, 32 * c: 32 * (c + 1)]
                nc.tensor.matmul(po_a[:, 0:256], sel, p2[:, 0:256],
                                 start=(c == 0), stop=(c == NCH - 1))
                nc.tensor.matmul(po_b[:, 256:512], sel, p2[:, 256:512],
                                 start=(c == 0), stop=(c == NCH - 1))

            o_a = work.tile([KL, 256], BF16, tag="oa")
            nc.scalar.copy(o_a[:], po_a[:, 0:256])
            nc.sync.dma_start(out_d.ap()[0:KL, :], o_a[:])
            o_b = work.tile([KL, 256], BF16, tag="ob")
            nc.vector.tensor_copy(o_b[:], po_b[:, 256:512])
            nc.scalar.dma_start(out_d.ap()[KL:2 * KL, :], o_b[:])

    nc.compile()
    return nc


def _host_prep(x_0, Vm, Vh):
    """Per-core input blobs: xin [8][128, XCOLS] bf16, vin [8][128, KVL]."""
    x_0 = np.ascontiguousarray(np.asarray(x_0), dtype=np.float32)
    vm = np.asarray(Vm)[:, 0].astype(np.float32)     # [HK, M, VK]
    vh = np.asarray(Vh)[:, 0].astype(np.float32)     # [HK, VK, H]

    vmf = vm.transpose(1, 0, 2).reshape(M, HK * VK)  # [m, (k,v)]
    vhf = vh.transpose(2, 0, 1).reshape(H, HK * VK)  # [h, (k,v)]

    # sel[c][p, j] = 1 iff j == 8*c + p//16   (kv partition p -> k column)
    sel = np.zeros((128, 4 * KL), dtype=np.float32)
    for c in range(NCH):
        for p in range(128):
            sel[p, 32 * c + 8 * c + p // 16] = 1.0

    selb = np.ascontiguousarray(sel.astype(BF))
    in_maps = []
    for core in range(NCORES):
        cb, ck = divmod(core, SK)
        shard = x_0[BL * cb:BL * (cb + 1)]                    # [BL, M, D]
        x0t = shard.transpose(1, 0, 2).reshape(M, BD)         # [m, (b,d)]
        xhrt = shard.reshape(BL, D, H).transpose(2, 0, 1).reshape(H, BD)
        ks = slice(KVL * ck, KVL * (ck + 1))
        xv = np.empty((128, 2 * BD), dtype=BF)
        xv[0:64, 0:BD] = x0t.astype(BF)
        xv[0:64, BD:] = vmf[:, ks].astype(BF)
        xv[64:128, 0:BD] = xhrt.astype(BF)
        xv[64:128, BD:] = vhf[:, ks].astype(BF)
        in_maps.append({"xv": np.ascontiguousarray(xv), "sel": selb})
    return in_maps


def run(x_0, x_h, Vm, Vh, **spmd_kwargs):
    in_maps = _host_prep(x_0, Vm, Vh)
    if "nc" not in _CACHE:
        _CACHE["nc"] = build_bass()
    nc = _CACHE["nc"]

    res = run_bass_kernel_spmd(nc, in_maps, core_ids=list(range(NCORES)),
                               **spmd_kwargs)
    # Unshard: per-core out is [k_loc, (b,d)] bf16 -> [BL, KL, D] f32
    full = np.empty((B, HK, D), dtype=np.float32)
    for core in range(NCORES):
        cb, ck = divmod(core, SK)
        o2 = np.asarray(res.results[core]["out"]).astype(np.float32)
        o = np.concatenate([o2[0:KL], o2[KL:2 * KL]], axis=1)  # [KL, BD]
        o = o.reshape(KL, BL, D).transpose(1, 0, 2)           # [BL, KL, D]
        full[BL * cb:BL * (cb + 1), KL * ck:KL * (ck + 1), :] = o
    return full, res


def kernel(x_0, x_h, Vm, Vh):
    return run(x_0, x_h, Vm, Vh)[0]


if __name__ == "__main__":
    rng = np.random.default_rng(0)
    x_0 = rng.standard_normal((B, M, D)).astype(np.float32)
    x_h = rng.standard_normal((B, H, D)).astype(np.float32)
    Vm = rng.standard_normal((HK, 1, M, VK)).astype(np.float32)
    Vh = rng.standard_normal((HK, 1, VK, H)).astype(np.float32)
    got = kernel(x_0, x_h, Vm, Vh)

    x0r = np.transpose(x_0, (0, 2, 1))
    xhr = x_0.reshape(B, D, H)
    a = np.einsum("bdi,kiv->bkdv", x0r, Vm[:, 0])
    bb = np.einsum("bdj,kvj->bkdv", xhr, Vh[:, 0])
    want = np.einsum("bkdv,bkdv->bkd", a, bb)
    err = np.abs(got - want).max() / np.abs(want).max()
    print("rel err:", err)
